# revision 58
# baseline (speedup 1.0000x reference)
"""Multi-head attention kernel for Trainium2 (8 NeuronCores).

Problem: B=2, S=2048, 16 heads, d_head=64, shared 64x64 per-head projections.
  out = softmax((q Wq^T)(k Wk^T)^T / 8) @ (v Wv^T), per (batch, head).

Sharding: 32 (b,h) pairs -> 4 pairs per core (data + head parallel).

Design (v2):
  All three 64x64 projections are folded into the inputs on the host:
  qt = qh (Wq^T Wk / 8), vp = vh Wv^T.  The device computes pure attention.

  Scores are computed transposed, S^T[k, q] = kh^T qt^T, as float32r
  matmuls (full-rate fp32, exact scores).  A 65th contraction row adds a
  constant bias B0 to every score so the DVE's exp can clamp at zero.

  The softmax exp is split across TWO engines working on disjoint groups of
  two k-tiles of the same score chunk (the v1 kernel ran all exps on ACT,
  which was the bottleneck):
    - ACT: true exp (bias -B0 undone via the activation bias AP) -> fp16 P
    - DVE: Schraudolph bit-trick exp: i16 = max(round(s * 1024*log2(e)), 0)
      bitcast fp16 (~2.5% rms on its ~3/8 share), one tensor_scalar instr.

  P@V runs in the [q, d] orientation: lhsT = P-tile [128k x 128q] fp16,
  rhs = v fp16 [128k x 65] (65th column of ones), accumulating oq[q, 4, 65]
  in PSUM across all 16 k-tiles.  The output lands directly in [q, d] layout
  with the softmax denominator in column 64 -- no transpose epilogue at all
  (v1 spent a PE transpose + PSUM->SBUF copy + per-block normalize).  Cost
  is only 65 cycles per matmul (output free size).

  Epilogue per chunk: one reciprocal [128,4], one broadcast tensor_tensor
  multiply, one store DMA.

  Every score group and P group gets its OWN pool tile: the tile framework
  tracks dependencies per tile, and slicing one big buffer serializes all
  consumers (measured 257us vs 104us for this exact kernel).

  P@V emission lags the exp stream by 2 groups so the PE's queued work only
  depends on exps that already completed (hides a cross-engine sem hop per
  group).  PSUM: 3 score tiles x 2 banks + 2 oq accumulators x 1 bank = 8.

  The output is stored in [r, t, d] layout (contiguous 1KB per partition;
  the [s, d] layout would need 256B descriptors, charged 2x by the DMA) and
  the host unshard absorbs the transpose.

  Timeline: v1 (ACT-only softmax, transpose epilogue) 144.3us ->
  v2 93.4us, rel_l2 vs the f32 reference 9.4e-3 (PE 90% busy at full
  clock; the rest is startup DMA latency, drain, ~50ns/group sem jitter).
"""

import numpy as np

import concourse.bacc as bacc
import concourse.mybir as mybir
import concourse.tile as tile
from concourse.bass_utils import run_bass_kernel_spmd
from concourse.masks import make_identity

F32 = mybir.dt.float32
F32R = mybir.dt.float32r
F16 = mybir.dt.float16
I16 = mybir.dt.int16

N_CORES = 8
B, S, D_EMBED = 2, 2048, 1024
N_HEADS = 16
D = 64
NPAIR = (B * N_HEADS) // N_CORES  # 4
NT = S // 128  # 16 k-tiles
NCHUNK = S // 512  # 4 q chunks
NG = 8  # exp groups per chunk (2 k-tiles each)

A16 = 1024.0 * np.log2(np.e)  # 1477.3197 (fp16 Schraudolph scale)
C_SCH = -59.0  # rms-centering of the Schraudolph interpolation error
B0 = (15360.0 + C_SCH) / A16  # score bias added via contraction row 64

# exp engine per group, by chunk parity: ACT is ~15% faster per element, so
# it takes 5 groups on even chunks and 4 on odd ones (4.52 would be even).
# The unavoidable same-engine adjacent pair sits mid-chunk (groups 2-4) where
# the OTHER engine runs the previous chunk's epilogue (recip/norm fillers),
# and chunk boundaries always alternate engines (...A | D... and ...D | A...).
PLAN_EVEN = ["ACT", "DVE", "ACT", "DVE", "ACT", "ACT", "DVE", "ACT"]
PLAN_ODD = ["DVE", "ACT", "DVE", "ACT", "DVE", "ACT", "ACT", "DVE"]
FILLER_START = 5  # first group index at which fillers are consumed

_NC_CACHE = {}


def build_nc(npair=NPAIR):
    nc = bacc.Bacc("TRN2", target_bir_lowering=False)
    qs_d = nc.dram_tensor("qs", [npair, D + 1, S], F32R, kind="ExternalInput").ap()
    ks_d = nc.dram_tensor("ks", [npair, D + 1, S], F32R, kind="ExternalInput").ap()
    vs_d = nc.dram_tensor("vs", [npair, 128, NT, D + 1], F16, kind="ExternalInput").ap()
    # output in [r, t, d] layout: contiguous 1KB-per-partition stores (the
    # [s, d] layout needs 256B descriptors, which DMA charges at 2x)
    out_d = nc.dram_tensor("out", [npair, 128, NT, D], F32, kind="ExternalOutput").ap()

    with tile.TileContext(nc) as tc:
        with (
            tc.tile_pool(name="const", bufs=1) as const,
            tc.tile_pool(name="io", bufs=2) as io,
            tc.tile_pool(name="pt_pool", bufs=6) as pt_pool,
            tc.tile_pool(name="st_pool", bufs=3, space="PSUM") as st_pool,
            tc.tile_pool(name="oq_pool", bufs=2, space="PSUM") as oq_pool,
        ):
            ident = const.tile([128, 128], F32)
            make_identity(nc, ident)
            bias_sb = const.tile([128, 1], F32)
            nc.vector.memset(bias_sb, -B0)

            pairs = {}

            def alloc_pair(p):
                st = {}
                st["qt"] = io.tile([D + 1, S], F32R, tag="qt", name=f"qt{p}")
                st["kh"] = io.tile([D + 1, S], F32R, tag="kh", name=f"kh{p}")
                st["v"] = io.tile([128, NT, D + 1], F16, tag="v", name=f"v{p}")
                pairs[p] = st

            def load_q(p, quarter):
                def run():
                    sl = slice(quarter * 512, (quarter + 1) * 512)
                    nc.sync.dma_start(out=pairs[p]["qt"][:, sl], in_=qs_d[p][:, sl])

                return run

            def load_k(p, quarter):
                def run():
                    sl = slice(quarter * 512, (quarter + 1) * 512)
                    nc.sync.dma_start(out=pairs[p]["kh"][:, sl], in_=ks_d[p][:, sl])

                return run

            def load_v(p, half):
                def run():
                    sl = slice(half * 8, half * 8 + 8)
                    nc.sync.dma_start(
                        out=pairs[p]["v"][:, sl, :], in_=vs_d[p][:, sl, :]
                    )

                return run

            # global score-group sequence: one (p, c, g) per exp group; the
            # score cursor runs 2 groups ahead of the exp cursor
            seq = [
                (p, c, g)
                for p in range(npair)
                for c in range(NCHUNK)
                for g in range(NG)
            ]
            st_tiles = {}
            cursor = [0]

            def ensure_scores(upto):
                while cursor[0] <= min(upto, len(seq) - 1):
                    p, c, g = seq[cursor[0]]
                    stt = pairs[p]
                    st = st_pool.tile(
                        [128, 2, 512], F32, tag="st", name=f"st{p}_{c}_{g}"
                    )
                    for j, t in enumerate((2 * g, 2 * g + 1)):
                        nc.tensor.matmul(
                            st[:, j, :],
                            stt["kh"][:, t * 128 : (t + 1) * 128],
                            stt["qt"][:, c * 512 : (c + 1) * 512],
                            start=True,
                            stop=True,
                        )
                    st_tiles[(p, c, g)] = st
                    cursor[0] += 1

            def emit_exp(p, c, g, eng):
                src = st_tiles.pop((p, c, g))
                pt = pt_pool.tile([128, 2, 512], I16, tag="pt", name=f"pt{p}_{c}_{g}")
                if eng == "ACT":
                    nc.scalar.activation(
                        pt.bitcast(F16),
                        src,
                        mybir.ActivationFunctionType.Exp,
                        bias=bias_sb[:, 0:1],
                    )
                else:
                    nc.vector.tensor_scalar(
                        pt,
                        src,
                        A16,
                        0.0,
                        mybir.AluOpType.mult,
                        mybir.AluOpType.max,
                    )
                return pt

            def emit_pv(p, c, g, pt, oq):
                # A start=True zeroes the ENTIRE PSUM bank, and all 4
                # q-subtiles share one bank -- so only the chunk's very first
                # P@V (t=0, sub=0) starts (zeroing the other subs' regions,
                # which hold stale data from 2 chunks ago); everything else
                # accumulates with start=False.
                v = pairs[p]["v"]
                for j, t in enumerate((2 * g, 2 * g + 1)):
                    for sub in range(4):
                        nc.tensor.matmul(
                            oq[:, sub, :],
                            pt[:, j, sub * 128 : (sub + 1) * 128].bitcast(F16),
                            v[:, t, :],
                            start=(t == 0 and sub == 0),
                            stop=(t == NT - 1 and sub == 3),
                            skip_group_check=True,
                        )

            def epilogue(p, c, oq):
                """Normalize chunk c of pair p and store. 3 pieces."""
                st = pairs[p]

                def recip():
                    st[f"rc{c % 2}"] = io.tile(
                        [128, 4], F32, tag="rc", name=f"rc{p}_{c}"
                    )
                    nc.vector.reciprocal(st[f"rc{c % 2}"], oq[:, :, D])

                def norm():
                    st[f"osb{c % 2}"] = io.tile(
                        [128, 4, D], F32, tag="osb", name=f"osb{p}_{c}"
                    )
                    rc = st[f"rc{c % 2}"]
                    nc.vector.tensor_tensor(
                        st[f"osb{c % 2}"],
                        oq[:, :, 0:D],
                        rc[:, :, None].broadcast_to([128, 4, D]),
                        mybir.AluOpType.mult,
                    )

                def store():
                    nc.sync.dma_start(
                        out=out_d[p][:, c * 4 : c * 4 + 4, :],
                        in_=st[f"osb{c % 2}"],
                    )

                return [recip, norm, store]

            pv_lag = []  # [(p, c, g, pt, oq)] emitted one group later, so the
            # PE's queued work after scores(g+2) depends on the PREVIOUS exp
            # (already done) instead of idling a sem hop on exp(g) every group

            def emit_chunk(p, c, fillers):
                oq = oq_pool.tile([128, 4, D + 1], F32, tag="oq", name=f"oq{p}_{c}")
                base = (p * NCHUNK + c) * NG
                plan = PLAN_EVEN if (p * NCHUNK + c) % 2 == 0 else PLAN_ODD
                fi = 0
                for g in range(NG):
                    ensure_scores(base + g + 2)
                    pt = emit_exp(p, c, g, plan[g])
                    if len(pv_lag) >= 2:
                        emit_pv(*pv_lag.pop(0))
                    pv_lag.append((p, c, g, pt, oq))
                    if g >= FILLER_START and fi < len(fillers):
                        fillers[fi]()
                        fi += 1
                while fi < len(fillers):
                    fillers[fi]()
                    fi += 1
                return oq

            # ---- software pipeline over pairs ----
            alloc_pair(0)
            # prologue loads: k/q quarter 0 on the SP queue; the rest via the
            # otherwise-idle Pool engine queue (SP SEQ serializes DMA dispatch
            # at 565ns each; Pool dispatches in ~36ns)
            # spread prologue dispatch across idle engine queues: each DMA
            # costs ~565-667ns of serial sequencer time on its issuing engine
            load_k(0, 0)()
            load_q(0, 0)()
            load_k(0, 1)()
            load_v(0, 0)()
            load_k(0, 2)()
            load_k(0, 3)()
            load_v(0, 1)()
            # warm the PE clock gate with dummy matmuls while the loads fly
            warm = oq_pool.tile([128, 4, D + 1], F32, tag="oq", name="warm")
            for w in range(10):
                nc.tensor.matmul(
                    warm[:, 0, :], ident[:, 0:128], ident[:, 0 : D + 1],
                    start=True, stop=True,
                )

            oq_prev = {}
            for p in range(npair):
                nxt = p + 1 if p + 1 < npair else None
                if nxt is not None and p == 0:
                    alloc_pair(nxt)
                for c in range(NCHUNK):
                    if p == 0 and c == 0:
                        fillers = [
                            load_q(0, 1),
                            load_q(0, 2),
                            load_q(0, 3),
                        ]
                        if nxt is not None:
                            fillers += [load_k(nxt, 0), load_k(nxt, 1)]
                    else:
                        prev = (p, c - 1) if c > 0 else (p - 1, NCHUNK - 1)
                        fillers = list(epilogue(*prev, oq_prev[prev]))
                        if nxt is not None:
                            if c == 0:
                                if p > 0:
                                    alloc_pair(nxt)
                                fillers += [load_k(nxt, 0), load_k(nxt, 1)]
                            elif c == 1:
                                fillers += [load_k(nxt, 2), load_k(nxt, 3)]
                            elif c == 2:
                                fillers += [
                                    load_q(nxt, 0),
                                    load_q(nxt, 1),
                                    load_v(nxt, 0),
                                ]
                            else:
                                fillers += [
                                    load_q(nxt, 2),
                                    load_q(nxt, 3),
                                    load_v(nxt, 1),
                                ]
                    oq_prev[(p, c)] = emit_chunk(p, c, fillers)
            while pv_lag:
                emit_pv(*pv_lag.pop(0))
            for piece in epilogue(
                npair - 1, NCHUNK - 1, oq_prev[(npair - 1, NCHUNK - 1)]
            ):
                piece()
            pairs.clear()
    nc.finalize()
    return nc


def _host_prep(k, q, v, Wk, Wq, Wv):
    m1 = ((Wq.T @ Wk) / np.sqrt(np.float32(D))).astype(np.float32)

    def split_heads(x):
        return x.reshape(B, S, N_HEADS, D).transpose(0, 2, 1, 3).reshape(-1, S, D)

    qh = split_heads(q)
    kh = split_heads(k)
    vh = split_heads(v)

    qt = (qh @ m1).astype(np.float32)  # [32, S, D]
    # transposed layouts with the bias contraction row 64
    qs = np.empty((B * N_HEADS, D + 1, S), dtype=np.float32)
    qs[:, :D, :] = qt.transpose(0, 2, 1)
    qs[:, D, :] = np.float32(B0 / 2.0)
    ks = np.empty((B * N_HEADS, D + 1, S), dtype=np.float32)
    ks[:, :D, :] = kh.transpose(0, 2, 1)
    ks[:, D, :] = np.float32(2.0)

    vp = (vh @ Wv.T).astype(np.float16)  # [32, S, D]
    vs = np.empty((B * N_HEADS, 128, NT, D + 1), dtype=np.float16)
    vs[:, :, :, :D] = vp.reshape(-1, NT, 128, D).transpose(0, 2, 1, 3)
    vs[:, :, :, D] = np.float16(1.0)
    return qs, ks, vs


def kernel(k, q, v, Wk, Wq, Wv):
    k = np.asarray(k, dtype=np.float32)
    q = np.asarray(q, dtype=np.float32)
    v = np.asarray(v, dtype=np.float32)
    Wk = np.asarray(Wk, dtype=np.float32)
    Wq = np.asarray(Wq, dtype=np.float32)
    Wv = np.asarray(Wv, dtype=np.float32)

    qs, ks, vs = _host_prep(k, q, v, Wk, Wq, Wv)

    if "nc" not in _NC_CACHE:
        _NC_CACHE["nc"] = build_nc()
    nc = _NC_CACHE["nc"]

    in_maps = []
    for c in range(N_CORES):
        sl = slice(c * NPAIR, (c + 1) * NPAIR)
        in_maps.append({"qs": qs[sl], "ks": ks[sl], "vs": vs[sl]})

    res = run_bass_kernel_spmd(nc, in_maps, core_ids=list(range(N_CORES)))
    outs = np.stack([r["out"] for r in res.results])  # [8, NPAIR, 128, NT, D]
    outs = outs.reshape(B, N_HEADS, 128, NT, D).transpose(0, 3, 2, 1, 4)
    out = outs.reshape(B, S, D_EMBED)  # s = t*128 + r
    return out


# revision 59
# speedup vs baseline: 1.0299x; 1.0299x over previous
"""Multi-head attention kernel for Trainium2 (8 NeuronCores).

Problem: B=2, S=2048, 16 heads, d_head=64, shared 64x64 per-head projections.
  out = softmax((q Wq^T)(k Wk^T)^T / 8) @ (v Wv^T), per (batch, head).

Sharding: 32 (b,h) pairs -> 4 pairs per core (data + head parallel).

Design (v2):
  All three 64x64 projections are folded into the inputs on the host:
  qt = qh (Wq^T Wk / 8), vp = vh Wv^T.  The device computes pure attention.

  Scores are computed transposed, S^T[k, q] = kh^T qt^T, as float32r
  matmuls (full-rate fp32, exact scores).  A 65th contraction row adds a
  constant bias B0 to every score so the DVE's exp can clamp at zero.

  The softmax exp is split across TWO engines working on disjoint groups of
  two k-tiles of the same score chunk (the v1 kernel ran all exps on ACT,
  which was the bottleneck):
    - ACT: true exp (bias -B0 undone via the activation bias AP) -> fp16 P
    - DVE: Schraudolph bit-trick exp: i16 = max(round(s * 1024*log2(e)), 0)
      bitcast fp16 (~2.5% rms on its ~3/8 share), one tensor_scalar instr.

  P@V runs in the [q, d] orientation: lhsT = P-tile [128k x 128q] fp16,
  rhs = v fp16 [128k x 65] (65th column of ones), accumulating oq[q, 4, 65]
  in PSUM across all 16 k-tiles.  The output lands directly in [q, d] layout
  with the softmax denominator in column 64 -- no transpose epilogue at all
  (v1 spent a PE transpose + PSUM->SBUF copy + per-block normalize).  Cost
  is only 65 cycles per matmul (output free size).

  Epilogue per chunk: one reciprocal [128,4], one broadcast tensor_tensor
  multiply, one store DMA.

  Every score group and P group gets its OWN pool tile: the tile framework
  tracks dependencies per tile, and slicing one big buffer serializes all
  consumers (measured 257us vs 104us for this exact kernel).

  P@V emission lags the exp stream by 2 groups so the PE's queued work only
  depends on exps that already completed (hides a cross-engine sem hop per
  group).  PSUM: 3 score tiles x 2 banks + 2 oq accumulators x 1 bank = 8.

  The output is stored in [r, t, d] layout (contiguous 1KB per partition;
  the [s, d] layout would need 256B descriptors, charged 2x by the DMA) and
  the host unshard absorbs the transpose.

  Timeline: v1 (ACT-only softmax, transpose epilogue) 144.3us ->
  v2 93.4us, rel_l2 vs the f32 reference 9.4e-3 (PE 90% busy at full
  clock; the rest is startup DMA latency, drain, ~50ns/group sem jitter).
"""

import numpy as np

import concourse.bacc as bacc
import concourse.mybir as mybir
import concourse.tile as tile
from concourse.bass_utils import run_bass_kernel_spmd
from concourse.masks import make_identity

F32 = mybir.dt.float32
F32R = mybir.dt.float32r
F16 = mybir.dt.float16
I16 = mybir.dt.int16

N_CORES = 8
B, S, D_EMBED = 2, 2048, 1024
N_HEADS = 16
D = 64
NPAIR = (B * N_HEADS) // N_CORES  # 4
NT = S // 128  # 16 k-tiles
NCHUNK = S // 512  # 4 q chunks
NG = 8  # exp groups per chunk (2 k-tiles each)

A16 = 1024.0 * np.log2(np.e)  # 1477.3197 (fp16 Schraudolph scale)
C_SCH = -59.0  # rms-centering of the Schraudolph interpolation error
B0 = (15360.0 + C_SCH) / A16  # score bias added via contraction row 64

# exp engine per group, by chunk parity: ACT is ~15% faster per element, so
# it takes 5 groups on even chunks and 4 on odd ones (4.52 would be even).
# The unavoidable same-engine adjacent pair sits mid-chunk (groups 2-4) where
# the OTHER engine runs the previous chunk's epilogue (recip/norm fillers),
# and chunk boundaries always alternate engines (...A | D... and ...D | A...).
PLAN_EVEN = ["ACT", "DVE", "ACT", "DVE", "ACT", "ACT", "DVE", "ACT"]
PLAN_ODD = ["DVE", "ACT", "DVE", "ACT", "DVE", "ACT", "ACT", "DVE"]
FILLER_START = 5  # first group index at which fillers are consumed

_NC_CACHE = {}


def build_nc(npair=NPAIR):
    nc = bacc.Bacc("TRN2", target_bir_lowering=False)
    qs_d = nc.dram_tensor("qs", [npair, D + 1, S], F32R, kind="ExternalInput").ap()
    ks_d = nc.dram_tensor("ks", [npair, D + 1, S], F32R, kind="ExternalInput").ap()
    vs_d = nc.dram_tensor("vs", [npair, 128, NT, D + 1], F16, kind="ExternalInput").ap()
    # output in [r, t, d] layout: contiguous 1KB-per-partition stores (the
    # [s, d] layout needs 256B descriptors, which DMA charges at 2x)
    out_d = nc.dram_tensor("out", [npair, 128, NT, D], F32, kind="ExternalOutput").ap()

    with tile.TileContext(nc) as tc:
        with (
            tc.tile_pool(name="const", bufs=1) as const,
            tc.tile_pool(name="io", bufs=2) as io,
            tc.tile_pool(name="pt_pool", bufs=5) as pt_pool,
            tc.tile_pool(name="st_pool", bufs=3, space="PSUM") as st_pool,
            tc.tile_pool(name="oq_pool", bufs=2, space="PSUM") as oq_pool,
        ):
            ident = const.tile([128, 128], F32)
            make_identity(nc, ident)
            bias_sb = const.tile([128, 1], F32)
            nc.vector.memset(bias_sb, -B0)

            pairs = {}

            def alloc_pair(p):
                st = {}
                st["qt"] = io.tile([D + 1, S], F32R, tag="qt", name=f"qt{p}")
                st["kh"] = io.tile([D + 1, S], F32R, tag="kh", name=f"kh{p}")
                st["v"] = io.tile([128, NT, D + 1], F16, tag="v", name=f"v{p}")
                pairs[p] = st

            def load_q(p, quarter):
                def run():
                    sl = slice(quarter * 512, (quarter + 1) * 512)
                    nc.sync.dma_start(out=pairs[p]["qt"][:, sl], in_=qs_d[p][:, sl])

                return run

            def load_k(p, quarter):
                def run():
                    sl = slice(quarter * 512, (quarter + 1) * 512)
                    nc.sync.dma_start(out=pairs[p]["kh"][:, sl], in_=ks_d[p][:, sl])

                return run

            def load_v(p, half):
                def run():
                    sl = slice(half * 8, half * 8 + 8)
                    nc.sync.dma_start(
                        out=pairs[p]["v"][:, sl, :], in_=vs_d[p][:, sl, :]
                    )

                return run

            # global score-group sequence: one (p, c, g) per exp group; the
            # score cursor runs 2 groups ahead of the exp cursor
            seq = [
                (p, c, g)
                for p in range(npair)
                for c in range(NCHUNK)
                for g in range(NG)
            ]
            st_tiles = {}
            cursor = [0]

            def ensure_scores(upto):
                while cursor[0] <= min(upto, len(seq) - 1):
                    p, c, g = seq[cursor[0]]
                    stt = pairs[p]
                    st = st_pool.tile(
                        [128, 2, 512], F32, tag="st", name=f"st{p}_{c}_{g}"
                    )
                    for j, t in enumerate((2 * g, 2 * g + 1)):
                        nc.tensor.matmul(
                            st[:, j, :],
                            stt["kh"][:, t * 128 : (t + 1) * 128],
                            stt["qt"][:, c * 512 : (c + 1) * 512],
                            start=True,
                            stop=True,
                        )
                    st_tiles[(p, c, g)] = st
                    cursor[0] += 1

            def emit_exp(p, c, g, eng):
                src = st_tiles.pop((p, c, g))
                pt = pt_pool.tile([128, 2, 512], I16, tag="pt", name=f"pt{p}_{c}_{g}")
                if eng == "ACT":
                    nc.scalar.activation(
                        pt.bitcast(F16),
                        src,
                        mybir.ActivationFunctionType.Exp,
                        bias=bias_sb[:, 0:1],
                    )
                else:
                    nc.vector.tensor_scalar(
                        pt,
                        src,
                        A16,
                        0.0,
                        mybir.AluOpType.mult,
                        mybir.AluOpType.max,
                    )
                return pt

            def emit_pv(p, c, g, pt, oq):
                # A start=True zeroes the ENTIRE PSUM bank, and all 4
                # q-subtiles share one bank -- so only the chunk's very first
                # P@V (t=0, sub=0) starts (zeroing the other subs' regions,
                # which hold stale data from 2 chunks ago); everything else
                # accumulates with start=False.
                v = pairs[p]["v"]
                for j, t in enumerate((2 * g, 2 * g + 1)):
                    for sub in range(4):
                        nc.tensor.matmul(
                            oq[:, sub, :],
                            pt[:, j, sub * 128 : (sub + 1) * 128].bitcast(F16),
                            v[:, t, :],
                            start=(t == 0 and sub == 0),
                            stop=(t == NT - 1 and sub == 3),
                            skip_group_check=True,
                        )

            def epilogue(p, c, oq):
                """Normalize chunk c of pair p and store. 3 pieces."""
                st = pairs[p]

                def recip():
                    st[f"rc{c % 2}"] = io.tile(
                        [128, 4], F32, tag="rc", name=f"rc{p}_{c}"
                    )
                    nc.vector.reciprocal(st[f"rc{c % 2}"], oq[:, :, D])

                def norm():
                    st[f"osb{c % 2}"] = io.tile(
                        [128, 4, D], F32, tag="osb", name=f"osb{p}_{c}"
                    )
                    rc = st[f"rc{c % 2}"]
                    nc.vector.tensor_tensor(
                        st[f"osb{c % 2}"],
                        oq[:, :, 0:D],
                        rc[:, :, None].broadcast_to([128, 4, D]),
                        mybir.AluOpType.mult,
                    )

                def store():
                    nc.sync.dma_start(
                        out=out_d[p][:, c * 4 : c * 4 + 4, :],
                        in_=st[f"osb{c % 2}"],
                    )

                return [recip, norm, store]

            pv_lag = []  # [(p, c, g, pt, oq)] emitted one group later, so the
            # PE's queued work after scores(g+2) depends on the PREVIOUS exp
            # (already done) instead of idling a sem hop on exp(g) every group

            def emit_chunk(p, c, fillers):
                oq = oq_pool.tile([128, 4, D + 1], F32, tag="oq", name=f"oq{p}_{c}")
                base = (p * NCHUNK + c) * NG
                plan = PLAN_EVEN if (p * NCHUNK + c) % 2 == 0 else PLAN_ODD
                fi = 0
                for g in range(NG):
                    ensure_scores(base + g + 2)
                    pt = emit_exp(p, c, g, plan[g])
                    if len(pv_lag) >= 2:
                        emit_pv(*pv_lag.pop(0))
                    pv_lag.append((p, c, g, pt, oq))
                    if g >= FILLER_START and fi < len(fillers):
                        fillers[fi]()
                        fi += 1
                while fi < len(fillers):
                    fillers[fi]()
                    fi += 1
                return oq

            # ---- software pipeline over pairs ----
            alloc_pair(0)
            # prologue loads: k/q quarter 0 on the SP queue; the rest via the
            # otherwise-idle Pool engine queue (SP SEQ serializes DMA dispatch
            # at 565ns each; Pool dispatches in ~36ns)
            # spread prologue dispatch across idle engine queues: each DMA
            # costs ~565-667ns of serial sequencer time on its issuing engine
            load_k(0, 0)()
            load_q(0, 0)()
            load_k(0, 1)()
            load_v(0, 0)()
            load_k(0, 2)()
            load_k(0, 3)()
            load_v(0, 1)()
            # warm the PE clock gate with dummy matmuls while the loads fly
            warm = oq_pool.tile([128, 4, D + 1], F32, tag="oq", name="warm")
            for w in range(10):
                nc.tensor.matmul(
                    warm[:, 0, :], ident[:, 0:128], ident[:, 0 : D + 1],
                    start=True, stop=True,
                )

            oq_prev = {}
            for p in range(npair):
                nxt = p + 1 if p + 1 < npair else None
                if nxt is not None and p == 0:
                    alloc_pair(nxt)
                for c in range(NCHUNK):
                    if p == 0 and c == 0:
                        fillers = [
                            load_q(0, 1),
                            load_q(0, 2),
                            load_q(0, 3),
                        ]
                        if nxt is not None:
                            fillers += [load_k(nxt, 0), load_k(nxt, 1)]
                    else:
                        prev = (p, c - 1) if c > 0 else (p - 1, NCHUNK - 1)
                        fillers = list(epilogue(*prev, oq_prev[prev]))
                        if nxt is not None:
                            if c == 0:
                                if p > 0:
                                    alloc_pair(nxt)
                                fillers += [load_k(nxt, 0), load_k(nxt, 1)]
                            elif c == 1:
                                fillers += [load_k(nxt, 2), load_k(nxt, 3)]
                            elif c == 2:
                                fillers += [
                                    load_q(nxt, 0),
                                    load_q(nxt, 1),
                                    load_v(nxt, 0),
                                ]
                            else:
                                fillers += [
                                    load_q(nxt, 2),
                                    load_q(nxt, 3),
                                    load_v(nxt, 1),
                                ]
                    oq_prev[(p, c)] = emit_chunk(p, c, fillers)
            while pv_lag:
                emit_pv(*pv_lag.pop(0))
            for piece in epilogue(
                npair - 1, NCHUNK - 1, oq_prev[(npair - 1, NCHUNK - 1)]
            ):
                piece()
            pairs.clear()
    nc.finalize()
    return nc


def _host_prep(k, q, v, Wk, Wq, Wv):
    m1 = ((Wq.T @ Wk) / np.sqrt(np.float32(D))).astype(np.float32)

    def split_heads(x):
        return x.reshape(B, S, N_HEADS, D).transpose(0, 2, 1, 3).reshape(-1, S, D)

    qh = split_heads(q)
    kh = split_heads(k)
    vh = split_heads(v)

    qt = (qh @ m1).astype(np.float32)  # [32, S, D]
    # transposed layouts with the bias contraction row 64
    qs = np.empty((B * N_HEADS, D + 1, S), dtype=np.float32)
    qs[:, :D, :] = qt.transpose(0, 2, 1)
    qs[:, D, :] = np.float32(B0 / 2.0)
    ks = np.empty((B * N_HEADS, D + 1, S), dtype=np.float32)
    ks[:, :D, :] = kh.transpose(0, 2, 1)
    ks[:, D, :] = np.float32(2.0)

    vp = (vh @ Wv.T).astype(np.float16)  # [32, S, D]
    vs = np.empty((B * N_HEADS, 128, NT, D + 1), dtype=np.float16)
    vs[:, :, :, :D] = vp.reshape(-1, NT, 128, D).transpose(0, 2, 1, 3)
    vs[:, :, :, D] = np.float16(1.0)
    return qs, ks, vs


def kernel(k, q, v, Wk, Wq, Wv):
    k = np.asarray(k, dtype=np.float32)
    q = np.asarray(q, dtype=np.float32)
    v = np.asarray(v, dtype=np.float32)
    Wk = np.asarray(Wk, dtype=np.float32)
    Wq = np.asarray(Wq, dtype=np.float32)
    Wv = np.asarray(Wv, dtype=np.float32)

    qs, ks, vs = _host_prep(k, q, v, Wk, Wq, Wv)

    if "nc" not in _NC_CACHE:
        _NC_CACHE["nc"] = build_nc()
    nc = _NC_CACHE["nc"]

    in_maps = []
    for c in range(N_CORES):
        sl = slice(c * NPAIR, (c + 1) * NPAIR)
        in_maps.append({"qs": qs[sl], "ks": ks[sl], "vs": vs[sl]})

    res = run_bass_kernel_spmd(nc, in_maps, core_ids=list(range(N_CORES)))
    outs = np.stack([r["out"] for r in res.results])  # [8, NPAIR, 128, NT, D]
    outs = outs.reshape(B, N_HEADS, 128, NT, D).transpose(0, 3, 2, 1, 4)
    out = outs.reshape(B, S, D_EMBED)  # s = t*128 + r
    return out


# revision 60
# speedup vs baseline: 1.0916x; 1.0600x over previous
"""Multi-head attention kernel for Trainium2 (8 NeuronCores).

Problem: B=2, S=2048, 16 heads, d_head=64, shared 64x64 per-head projections.
  out = softmax((q Wq^T)(k Wk^T)^T / 8) @ (v Wv^T), per (batch, head).

Sharding: 32 (b,h) pairs -> 4 pairs per core (data + head parallel).

Design (v2):
  All three 64x64 projections are folded into the inputs on the host:
  qt = qh (Wq^T Wk / 8), vp = vh Wv^T.  The device computes pure attention.

  Scores are computed transposed, S^T[k, q] = kh^T qt^T, as float32r
  matmuls (full-rate fp32, exact scores).  A 65th contraction row adds a
  constant bias B0 to every score so the DVE's exp can clamp at zero.

  The softmax exp is split across TWO engines working on disjoint groups of
  two k-tiles of the same score chunk (the v1 kernel ran all exps on ACT,
  which was the bottleneck):
    - ACT: true exp (bias -B0 undone via the activation bias AP) -> fp16 P
    - DVE: Schraudolph bit-trick exp: i16 = max(round(s * 1024*log2(e)), 0)
      bitcast fp16 (~2.5% rms on its ~3/8 share), one tensor_scalar instr.

  P@V runs in the [q, d] orientation: lhsT = P-tile [128k x 128q] fp16,
  rhs = v fp16 [128k x 65] (65th column of ones), accumulating oq[q, 4, 65]
  in PSUM across all 16 k-tiles.  The output lands directly in [q, d] layout
  with the softmax denominator in column 64 -- no transpose epilogue at all
  (v1 spent a PE transpose + PSUM->SBUF copy + per-block normalize).  Cost
  is only 65 cycles per matmul (output free size).

  Epilogue per chunk: one reciprocal [128,4], one broadcast tensor_tensor
  multiply, one store DMA.

  Every score group and P group gets its OWN pool tile: the tile framework
  tracks dependencies per tile, and slicing one big buffer serializes all
  consumers (measured 257us vs 104us for this exact kernel).

  P@V emission lags the exp stream by 2 groups so the PE's queued work only
  depends on exps that already completed (hides a cross-engine sem hop per
  group).  PSUM: 3 score tiles x 2 banks + 2 oq accumulators x 1 bank = 8.

  The output is stored in [r, t, d] layout (contiguous 1KB per partition;
  the [s, d] layout would need 256B descriptors, charged 2x by the DMA) and
  the host unshard absorbs the transpose.

  Timeline: v1 (ACT-only softmax, transpose epilogue) 144.3us ->
  v2 93.4us, rel_l2 vs the f32 reference 9.4e-3 (PE 90% busy at full
  clock; the rest is startup DMA latency, drain, ~50ns/group sem jitter).
"""

import numpy as np

import concourse.bacc as bacc
import concourse.mybir as mybir
import concourse.tile as tile
from concourse.bass_utils import run_bass_kernel_spmd
from concourse.masks import make_identity

F32 = mybir.dt.float32
F32R = mybir.dt.float32r
F16 = mybir.dt.float16
F8E4 = mybir.dt.float8e4
I16 = mybir.dt.int16

N_CORES = 8
B, S, D_EMBED = 2, 2048, 1024
N_HEADS = 16
D = 64
NPAIR = (B * N_HEADS) // N_CORES  # 4
NT = S // 128  # 16 k-tiles
NCHUNK = S // 512  # 4 q chunks
NG = 8  # exp groups per chunk (2 k-tiles each)

A16 = 1024.0 * np.log2(np.e)  # 1477.3197 (fp16 Schraudolph scale)
QS = 16.0  # host scale on qt so e4m3 stays in normal range
# score bias via contraction row 96 (both DoubleRow halves, e4m3-exact
# factors): 16*10 + 2*2.875 = 165.75 in 16x-score units, which lands the
# fp16 Schraudolph centering at c = 165.75*A16/16 - 15360 = -55.9 (~optimal)
BIAS16 = 165.75

# exp engine per group, by chunk parity: ACT is ~15% faster per element, so
# it takes 5 groups on even chunks and 4 on odd ones (4.52 would be even).
# The unavoidable same-engine adjacent pair sits mid-chunk (groups 2-4) where
# the OTHER engine runs the previous chunk's epilogue (recip/norm fillers),
# and chunk boundaries always alternate engines (...A | D... and ...D | A...).
PLAN_EVEN = ["ACT", "DVE", "ACT", "DVE", "ACT", "ACT", "DVE", "ACT"]
PLAN_ODD = ["DVE", "ACT", "DVE", "ACT", "DVE", "ACT", "ACT", "DVE"]
FILLER_START = 5  # first group index at which fillers are consumed

_NC_CACHE = {}


def build_nc(npair=NPAIR):
    nc = bacc.Bacc("TRN2", target_bir_lowering=False)
    # two-term fp8 score operands: rows 0-31 hi-part (d-split), 32-63 and
    # 64-95 the hi*lo cross terms, row 96 the bias pair.  One [97,2]-DoubleRow
    # matmul per k-tile computes k_hi q_hi + k_hi q_lo + k_lo q_hi + bias
    # at 0.5 cyc/row (half of f32r) with ~bf16 accuracy (lo*lo dropped).
    qs_d = nc.dram_tensor("qs", [npair, 97, 2, S], F8E4, kind="ExternalInput").ap()
    ks_d = nc.dram_tensor("ks", [npair, 97, 2, S], F8E4, kind="ExternalInput").ap()
    vs_d = nc.dram_tensor("vs", [npair, 128, NT, D + 1], F16, kind="ExternalInput").ap()
    # output in [r, t, d] layout: contiguous 1KB-per-partition stores (the
    # [s, d] layout needs 256B descriptors, which DMA charges at 2x)
    out_d = nc.dram_tensor("out", [npair, 128, NT, D], F32, kind="ExternalOutput").ap()

    with tile.TileContext(nc) as tc:
        with (
            tc.tile_pool(name="const", bufs=1) as const,
            tc.tile_pool(name="io", bufs=2) as io,
            tc.tile_pool(name="pt_pool", bufs=5) as pt_pool,
            tc.tile_pool(name="st_pool", bufs=3, space="PSUM") as st_pool,
            tc.tile_pool(name="oq_pool", bufs=2, space="PSUM") as oq_pool,
        ):
            ident = const.tile([128, 128], F32)
            make_identity(nc, ident)
            bias_sb = const.tile([128, 1], F32)
            nc.vector.memset(bias_sb, -BIAS16 / QS)

            pairs = {}

            def alloc_pair(p):
                st = {}
                st["qt"] = io.tile([97, 2, S], F8E4, tag="qt", name=f"qt{p}")
                st["kh"] = io.tile([97, 2, S], F8E4, tag="kh", name=f"kh{p}")
                st["v"] = io.tile([128, NT, D + 1], F16, tag="v", name=f"v{p}")
                pairs[p] = st

            def load_q(p, quarter):
                def run():
                    sl = slice(quarter * 512, (quarter + 1) * 512)
                    nc.sync.dma_start(
                        out=pairs[p]["qt"][:, :, sl], in_=qs_d[p][:, :, sl]
                    )

                return run

            def load_k(p, quarter):
                def run():
                    sl = slice(quarter * 512, (quarter + 1) * 512)
                    nc.sync.dma_start(
                        out=pairs[p]["kh"][:, :, sl], in_=ks_d[p][:, :, sl]
                    )

                return run

            def load_v(p, half):
                def run():
                    sl = slice(half * 8, half * 8 + 8)
                    nc.sync.dma_start(
                        out=pairs[p]["v"][:, sl, :], in_=vs_d[p][:, sl, :]
                    )

                return run

            # global score-group sequence: one (p, c, g) per exp group; the
            # score cursor runs 2 groups ahead of the exp cursor
            seq = [
                (p, c, g)
                for p in range(npair)
                for c in range(NCHUNK)
                for g in range(NG)
            ]
            st_tiles = {}
            cursor = [0]

            def ensure_scores(upto):
                while cursor[0] <= min(upto, len(seq) - 1):
                    p, c, g = seq[cursor[0]]
                    stt = pairs[p]
                    st = st_pool.tile(
                        [128, 2, 512], F32, tag="st", name=f"st{p}_{c}_{g}"
                    )
                    for j, t in enumerate((2 * g, 2 * g + 1)):
                        nc.tensor.matmul(
                            st[:, j, :],
                            stt["kh"][:, :, t * 128 : (t + 1) * 128],
                            stt["qt"][:, :, c * 512 : (c + 1) * 512],
                            start=True,
                            stop=True,
                            perf_mode=mybir.MatmulPerfMode.DoubleRow,
                        )
                    st_tiles[(p, c, g)] = st
                    cursor[0] += 1

            def emit_exp(p, c, g, eng):
                src = st_tiles.pop((p, c, g))
                pt = pt_pool.tile([128, 2, 512], I16, tag="pt", name=f"pt{p}_{c}_{g}")
                if eng == "ACT":
                    nc.scalar.activation(
                        pt.bitcast(F16),
                        src,
                        mybir.ActivationFunctionType.Exp,
                        bias=bias_sb[:, 0:1],
                        scale=1.0 / QS,
                    )
                else:
                    nc.vector.tensor_scalar(
                        pt,
                        src,
                        A16 / QS,
                        0.0,
                        mybir.AluOpType.mult,
                        mybir.AluOpType.max,
                    )
                return pt

            def emit_pv(p, c, g, pt, oq):
                # A start=True zeroes the ENTIRE PSUM bank, and all 4
                # q-subtiles share one bank -- so only the chunk's very first
                # P@V (t=0, sub=0) starts (zeroing the other subs' regions,
                # which hold stale data from 2 chunks ago); everything else
                # accumulates with start=False.
                v = pairs[p]["v"]
                for j, t in enumerate((2 * g, 2 * g + 1)):
                    for sub in range(4):
                        nc.tensor.matmul(
                            oq[:, sub, :],
                            pt[:, j, sub * 128 : (sub + 1) * 128].bitcast(F16),
                            v[:, t, :],
                            start=(t == 0 and sub == 0),
                            stop=(t == NT - 1 and sub == 3),
                            skip_group_check=True,
                        )

            def epilogue(p, c, oq):
                """Normalize chunk c of pair p and store. 3 pieces."""
                st = pairs[p]

                def recip():
                    st[f"rc{c % 2}"] = io.tile(
                        [128, 4], F32, tag="rc", name=f"rc{p}_{c}"
                    )
                    nc.vector.reciprocal(st[f"rc{c % 2}"], oq[:, :, D])

                def norm():
                    st[f"osb{c % 2}"] = io.tile(
                        [128, 4, D], F32, tag="osb", name=f"osb{p}_{c}"
                    )
                    rc = st[f"rc{c % 2}"]
                    nc.vector.tensor_tensor(
                        st[f"osb{c % 2}"],
                        oq[:, :, 0:D],
                        rc[:, :, None].broadcast_to([128, 4, D]),
                        mybir.AluOpType.mult,
                    )

                def store():
                    nc.sync.dma_start(
                        out=out_d[p][:, c * 4 : c * 4 + 4, :],
                        in_=st[f"osb{c % 2}"],
                    )

                return [recip, norm, store]

            pv_lag = []  # [(p, c, g, pt, oq)] emitted one group later, so the
            # PE's queued work after scores(g+2) depends on the PREVIOUS exp
            # (already done) instead of idling a sem hop on exp(g) every group

            def emit_chunk(p, c, fillers):
                oq = oq_pool.tile([128, 4, D + 1], F32, tag="oq", name=f"oq{p}_{c}")
                base = (p * NCHUNK + c) * NG
                plan = PLAN_EVEN if (p * NCHUNK + c) % 2 == 0 else PLAN_ODD
                fi = 0
                for g in range(NG):
                    ensure_scores(base + g + 2)
                    pt = emit_exp(p, c, g, plan[g])
                    if len(pv_lag) >= 2:
                        emit_pv(*pv_lag.pop(0))
                    pv_lag.append((p, c, g, pt, oq))
                    if g >= FILLER_START and fi < len(fillers):
                        fillers[fi]()
                        fi += 1
                while fi < len(fillers):
                    fillers[fi]()
                    fi += 1
                return oq

            # ---- software pipeline over pairs ----
            alloc_pair(0)
            # prologue loads: k/q quarter 0 on the SP queue; the rest via the
            # otherwise-idle Pool engine queue (SP SEQ serializes DMA dispatch
            # at 565ns each; Pool dispatches in ~36ns)
            # spread prologue dispatch across idle engine queues: each DMA
            # costs ~565-667ns of serial sequencer time on its issuing engine
            load_k(0, 0)()
            load_q(0, 0)()
            load_k(0, 1)()
            load_v(0, 0)()
            load_k(0, 2)()
            load_k(0, 3)()
            load_v(0, 1)()
            # warm the PE clock gate with dummy matmuls while the loads fly
            warm = oq_pool.tile([128, 4, D + 1], F32, tag="oq", name="warm")
            for w in range(10):
                nc.tensor.matmul(
                    warm[:, 0, :], ident[:, 0:128], ident[:, 0 : D + 1],
                    start=True, stop=True,
                )

            oq_prev = {}
            for p in range(npair):
                nxt = p + 1 if p + 1 < npair else None
                if nxt is not None and p == 0:
                    alloc_pair(nxt)
                for c in range(NCHUNK):
                    if p == 0 and c == 0:
                        fillers = [
                            load_q(0, 1),
                            load_q(0, 2),
                            load_q(0, 3),
                        ]
                        if nxt is not None:
                            fillers += [load_k(nxt, 0), load_k(nxt, 1)]
                    else:
                        prev = (p, c - 1) if c > 0 else (p - 1, NCHUNK - 1)
                        fillers = list(epilogue(*prev, oq_prev[prev]))
                        if nxt is not None:
                            if c == 0:
                                if p > 0:
                                    alloc_pair(nxt)
                                fillers += [load_k(nxt, 0), load_k(nxt, 1)]
                            elif c == 1:
                                fillers += [load_k(nxt, 2), load_k(nxt, 3)]
                            elif c == 2:
                                fillers += [
                                    load_q(nxt, 0),
                                    load_q(nxt, 1),
                                    load_v(nxt, 0),
                                ]
                            else:
                                fillers += [
                                    load_q(nxt, 2),
                                    load_q(nxt, 3),
                                    load_v(nxt, 1),
                                ]
                    oq_prev[(p, c)] = emit_chunk(p, c, fillers)
            while pv_lag:
                emit_pv(*pv_lag.pop(0))
            for piece in epilogue(
                npair - 1, NCHUNK - 1, oq_prev[(npair - 1, NCHUNK - 1)]
            ):
                piece()
            pairs.clear()
    nc.finalize()
    return nc


def _host_prep(k, q, v, Wk, Wq, Wv):
    m1 = ((Wq.T @ Wk) / np.sqrt(np.float32(D))).astype(np.float32)

    def split_heads(x):
        return x.reshape(B, S, N_HEADS, D).transpose(0, 2, 1, 3).reshape(-1, S, D)

    qh = split_heads(q)
    kh = split_heads(k)
    vh = split_heads(v)

    import ml_dtypes

    E4 = ml_dtypes.float8_e4m3
    qt = (qh @ m1).astype(np.float32) * np.float32(QS)  # [32, S, D]

    def hi_lo(x):
        hi = x.astype(E4)
        lo = (x - hi.astype(np.float32)).astype(E4)
        return hi, lo

    def dsplit(x):  # [P, S, D] -> [P, 32, 2, S]: [p, t] <- d = t*32 + p
        return x.transpose(0, 2, 1).reshape(-1, 2, 32, S).transpose(0, 2, 1, 3)

    q_hi, q_lo = hi_lo(qt)
    k_hi, k_lo = hi_lo(kh.astype(np.float32))
    qs = np.zeros((B * N_HEADS, 97, 2, S), dtype=E4)
    ks = np.zeros((B * N_HEADS, 97, 2, S), dtype=E4)
    qs[:, 0:32] = dsplit(q_hi.astype(np.float32))
    qs[:, 32:64] = dsplit(q_lo.astype(np.float32))
    qs[:, 64:96] = dsplit(q_hi.astype(np.float32))
    ks[:, 0:32] = dsplit(k_hi.astype(np.float32))
    ks[:, 32:64] = dsplit(k_hi.astype(np.float32))
    ks[:, 64:96] = dsplit(k_lo.astype(np.float32))
    qs[:, 96, 0, :] = E4(10.0)
    qs[:, 96, 1, :] = E4(2.875)
    ks[:, 96, 0, :] = E4(16.0)
    ks[:, 96, 1, :] = E4(2.0)

    vp = (vh @ Wv.T).astype(np.float16)  # [32, S, D]
    vs = np.empty((B * N_HEADS, 128, NT, D + 1), dtype=np.float16)
    vs[:, :, :, :D] = vp.reshape(-1, NT, 128, D).transpose(0, 2, 1, 3)
    vs[:, :, :, D] = np.float16(1.0)
    return qs, ks, vs


def kernel(k, q, v, Wk, Wq, Wv):
    k = np.asarray(k, dtype=np.float32)
    q = np.asarray(q, dtype=np.float32)
    v = np.asarray(v, dtype=np.float32)
    Wk = np.asarray(Wk, dtype=np.float32)
    Wq = np.asarray(Wq, dtype=np.float32)
    Wv = np.asarray(Wv, dtype=np.float32)

    qs, ks, vs = _host_prep(k, q, v, Wk, Wq, Wv)

    if "nc" not in _NC_CACHE:
        _NC_CACHE["nc"] = build_nc()
    nc = _NC_CACHE["nc"]

    in_maps = []
    for c in range(N_CORES):
        sl = slice(c * NPAIR, (c + 1) * NPAIR)
        in_maps.append({"qs": qs[sl], "ks": ks[sl], "vs": vs[sl]})

    res = run_bass_kernel_spmd(nc, in_maps, core_ids=list(range(N_CORES)))
    outs = np.stack([r["out"] for r in res.results])  # [8, NPAIR, 128, NT, D]
    outs = outs.reshape(B, N_HEADS, 128, NT, D).transpose(0, 3, 2, 1, 4)
    out = outs.reshape(B, S, D_EMBED)  # s = t*128 + r
    return out


# revision 65
# speedup vs baseline: 1.0923x; 1.0006x over previous
"""Multi-head attention kernel for Trainium2 (8 NeuronCores).

Problem: B=2, S=2048, 16 heads, d_head=64, shared 64x64 per-head projections.
  out = softmax((q Wq^T)(k Wk^T)^T / 8) @ (v Wv^T), per (batch, head).

Sharding: 32 (b,h) pairs -> 4 pairs per core (data + head parallel).

Design (v2):
  All three 64x64 projections are folded into the inputs on the host:
  qt = qh (Wq^T Wk / 8), vp = vh Wv^T.  The device computes pure attention.

  Scores are computed transposed, S^T[k, q] = kh^T qt^T, as ONE fp8
  DoubleRow matmul per k-tile (0.5 cyc/row, HALF of f32r): q and k are
  split host-side into e4m3 hi+lo terms and the [97, 2] contraction packs
  k_hi q_hi + k_hi q_lo + k_lo q_hi (the dropped lo*lo term is ~3e-4
  relative, ~bf16 score accuracy).  Row 96 packs a constant bias into both
  DoubleRow halves (16*10 + 2*2.875 = 165.75, all e4m3-exact) so the DVE's
  exp can clamp at zero and the Schraudolph centering lands near-optimal.

  The softmax exp is split across TWO engines working on disjoint groups of
  two k-tiles of the same score chunk (the v1 kernel ran all exps on ACT,
  which was the bottleneck):
    - ACT: true exp (bias -B0 undone via the activation bias AP) -> fp16 P
    - DVE: Schraudolph bit-trick exp: i16 = max(round(s * 1024*log2(e)), 0)
      bitcast fp16 (~2.5% rms on its ~3/8 share), one tensor_scalar instr.

  P@V runs in the [q, d] orientation: lhsT = P-tile [128k x 128q] fp16,
  rhs = v fp16 [128k x 65] (65th column of ones), accumulating oq[q, 4, 65]
  in PSUM across all 16 k-tiles.  The output lands directly in [q, d] layout
  with the softmax denominator in column 64 -- no transpose epilogue at all
  (v1 spent a PE transpose + PSUM->SBUF copy + per-block normalize).  Cost
  is only 65 cycles per matmul (output free size).

  Epilogue per chunk: one reciprocal [128,4], one broadcast tensor_tensor
  multiply, one store DMA.

  Every score group and P group gets its OWN pool tile: the tile framework
  tracks dependencies per tile, and slicing one big buffer serializes all
  consumers (measured 257us vs 104us for this exact kernel).

  P@V emission lags the exp stream by 2 groups so the PE's queued work only
  depends on exps that already completed (hides a cross-engine sem hop per
  group).  PSUM: 3 score tiles x 2 banks + 2 oq accumulators x 1 bank = 8.

  The output is stored in [r, t, d] layout (contiguous 1KB per partition;
  the [s, d] layout would need 256B descriptors, charged 2x by the DMA) and
  the host unshard absorbs the transpose.

  Timeline: v1 (ACT-only softmax, transpose epilogue) 144.3us ->
  f32r-score v2 93.4us -> fp8-two-term-score v3 88.1us, rel_l2 vs the
  f32 reference 9.6e-3.  ACT/DVE (the softmax) are now the bottleneck at
  ~86% busy; PE at 65%.  The rest is startup DMA latency and drain.
"""

import numpy as np

import concourse.bacc as bacc
import concourse.mybir as mybir
import concourse.tile as tile
from concourse.bass_utils import run_bass_kernel_spmd
from concourse.masks import make_identity

F32 = mybir.dt.float32
F32R = mybir.dt.float32r
F16 = mybir.dt.float16
F8E4 = mybir.dt.float8e4
I16 = mybir.dt.int16

N_CORES = 8
B, S, D_EMBED = 2, 2048, 1024
N_HEADS = 16
D = 64
NPAIR = (B * N_HEADS) // N_CORES  # 4
NT = S // 128  # 16 k-tiles
NCHUNK = S // 512  # 4 q chunks
NG = 8  # exp groups per chunk (2 k-tiles each)

A16 = 1024.0 * np.log2(np.e)  # 1477.3197 (fp16 Schraudolph scale)
QS = 16.0  # host scale on qt so e4m3 stays in normal range
# score bias via contraction row 96 (both DoubleRow halves, e4m3-exact
# factors): 16*10 + 2*2.875 = 165.75 in 16x-score units, which lands the
# fp16 Schraudolph centering at c = 165.75*A16/16 - 15360 = -55.9 (~optimal)
BIAS16 = 165.75

# exp engine per group, by chunk parity: ACT is ~15% faster per element, so
# it takes 5 groups on even chunks and 4 on odd ones (4.52 would be even).
# The unavoidable same-engine adjacent pair sits mid-chunk (groups 2-4) where
# the OTHER engine runs the previous chunk's epilogue (recip/norm fillers),
# and chunk boundaries always alternate engines (...A | D... and ...D | A...).
PLAN_EVEN = ["ACT", "DVE", "ACT", "DVE", "ACT", "ACT", "DVE", "ACT"]
PLAN_ODD = ["DVE", "ACT", "DVE", "ACT", "DVE", "ACT", "ACT", "DVE"]
FILLER_START = 2  # first group index at which fillers are consumed

_NC_CACHE = {}


def build_nc(npair=NPAIR):
    nc = bacc.Bacc("TRN2", target_bir_lowering=False)
    # two-term fp8 score operands: rows 0-31 hi-part (d-split), 32-63 and
    # 64-95 the hi*lo cross terms, row 96 the bias pair.  One [97,2]-DoubleRow
    # matmul per k-tile computes k_hi q_hi + k_hi q_lo + k_lo q_hi + bias
    # at 0.5 cyc/row (half of f32r) with ~bf16 accuracy (lo*lo dropped).
    qs_d = nc.dram_tensor("qs", [npair, 97, 2, S], F8E4, kind="ExternalInput").ap()
    ks_d = nc.dram_tensor("ks", [npair, 97, 2, S], F8E4, kind="ExternalInput").ap()
    vs_d = nc.dram_tensor("vs", [npair, 128, NT, D + 1], F16, kind="ExternalInput").ap()
    # output in [r, t, d] layout: contiguous 1KB-per-partition stores (the
    # [s, d] layout needs 256B descriptors, which DMA charges at 2x)
    out_d = nc.dram_tensor("out", [npair, 128, NT, D], F32, kind="ExternalOutput").ap()

    with tile.TileContext(nc) as tc:
        with (
            tc.tile_pool(name="const", bufs=1) as const,
            tc.tile_pool(name="io", bufs=2) as io,
            tc.tile_pool(name="pt_pool", bufs=5) as pt_pool,
            tc.tile_pool(name="st_pool", bufs=3, space="PSUM") as st_pool,
            tc.tile_pool(name="oq_pool", bufs=2, space="PSUM") as oq_pool,
        ):
            ident = const.tile([128, 128], F32)
            make_identity(nc, ident)
            bias_sb = const.tile([128, 1], F32)
            nc.vector.memset(bias_sb, -BIAS16 / QS)

            pairs = {}

            def alloc_pair(p):
                st = {}
                st["qt"] = io.tile([97, 2, S], F8E4, tag="qt", name=f"qt{p}")
                st["kh"] = io.tile([97, 2, S], F8E4, tag="kh", name=f"kh{p}")
                st["v"] = io.tile([128, NT, D + 1], F16, tag="v", name=f"v{p}")
                pairs[p] = st

            def load_q(p, quarter):
                def run():
                    sl = slice(quarter * 512, (quarter + 1) * 512)
                    nc.sync.dma_start(
                        out=pairs[p]["qt"][:, :, sl], in_=qs_d[p][:, :, sl]
                    )

                return run

            def load_k(p, quarter):
                def run():
                    sl = slice(quarter * 512, (quarter + 1) * 512)
                    nc.sync.dma_start(
                        out=pairs[p]["kh"][:, :, sl], in_=ks_d[p][:, :, sl]
                    )

                return run

            def load_v(p, half):
                def run():
                    sl = slice(half * 8, half * 8 + 8)
                    nc.sync.dma_start(
                        out=pairs[p]["v"][:, sl, :], in_=vs_d[p][:, sl, :]
                    )

                return run

            # global score-group sequence: one (p, c, g) per exp group; the
            # score cursor runs 2 groups ahead of the exp cursor
            seq = [
                (p, c, g)
                for p in range(npair)
                for c in range(NCHUNK)
                for g in range(NG)
            ]
            st_tiles = {}
            cursor = [0]

            def ensure_scores(upto):
                while cursor[0] <= min(upto, len(seq) - 1):
                    p, c, g = seq[cursor[0]]
                    stt = pairs[p]
                    st = st_pool.tile(
                        [128, 2, 512], F32, tag="st", name=f"st{p}_{c}_{g}"
                    )
                    for j, t in enumerate((2 * g, 2 * g + 1)):
                        nc.tensor.matmul(
                            st[:, j, :],
                            stt["kh"][:, :, t * 128 : (t + 1) * 128],
                            stt["qt"][:, :, c * 512 : (c + 1) * 512],
                            start=True,
                            stop=True,
                            perf_mode=mybir.MatmulPerfMode.DoubleRow,
                        )
                    st_tiles[(p, c, g)] = st
                    cursor[0] += 1

            def emit_exp(p, c, g, eng):
                src = st_tiles.pop((p, c, g))
                pt = pt_pool.tile([128, 2, 512], I16, tag="pt", name=f"pt{p}_{c}_{g}")
                if eng == "ACT":
                    nc.scalar.activation(
                        pt.bitcast(F16),
                        src,
                        mybir.ActivationFunctionType.Exp,
                        bias=bias_sb[:, 0:1],
                        scale=1.0 / QS,
                    )
                else:
                    nc.vector.tensor_scalar(
                        pt,
                        src,
                        A16 / QS,
                        0.0,
                        mybir.AluOpType.mult,
                        mybir.AluOpType.max,
                    )
                return pt

            def emit_pv(p, c, g, pt, oq):
                # A start=True zeroes the ENTIRE PSUM bank, and all 4
                # q-subtiles share one bank -- so only the chunk's very first
                # P@V (t=0, sub=0) starts (zeroing the other subs' regions,
                # which hold stale data from 2 chunks ago); everything else
                # accumulates with start=False.
                v = pairs[p]["v"]
                for j, t in enumerate((2 * g, 2 * g + 1)):
                    for sub in range(4):
                        nc.tensor.matmul(
                            oq[:, sub, :],
                            pt[:, j, sub * 128 : (sub + 1) * 128].bitcast(F16),
                            v[:, t, :],
                            start=(t == 0 and sub == 0),
                            stop=(t == NT - 1 and sub == 3),
                            skip_group_check=True,
                        )

            def epilogue(p, c, oq):
                """Normalize chunk c of pair p and store. 3 pieces."""
                st = pairs[p]

                def recip():
                    st[f"rc{c % 2}"] = io.tile(
                        [128, 4], F32, tag="rc", name=f"rc{p}_{c}"
                    )
                    nc.vector.reciprocal(st[f"rc{c % 2}"], oq[:, :, D])

                def norm():
                    st[f"osb{c % 2}"] = io.tile(
                        [128, 4, D], F32, tag="osb", name=f"osb{p}_{c}"
                    )
                    rc = st[f"rc{c % 2}"]
                    nc.vector.tensor_tensor(
                        st[f"osb{c % 2}"],
                        oq[:, :, 0:D],
                        rc[:, :, None].broadcast_to([128, 4, D]),
                        mybir.AluOpType.mult,
                    )

                def store():
                    nc.sync.dma_start(
                        out=out_d[p][:, c * 4 : c * 4 + 4, :],
                        in_=st[f"osb{c % 2}"],
                    )

                return [recip, norm, store]

            pv_lag = []  # [(p, c, g, pt, oq)] emitted one group later, so the
            # PE's queued work after scores(g+2) depends on the PREVIOUS exp
            # (already done) instead of idling a sem hop on exp(g) every group

            def emit_chunk(p, c, fillers):
                oq = oq_pool.tile([128, 4, D + 1], F32, tag="oq", name=f"oq{p}_{c}")
                base = (p * NCHUNK + c) * NG
                plan = PLAN_EVEN if (p * NCHUNK + c) % 2 == 0 else PLAN_ODD
                fi = 0
                for g in range(NG):
                    ensure_scores(base + g + 2)
                    pt = emit_exp(p, c, g, plan[g])
                    if len(pv_lag) >= 2:
                        emit_pv(*pv_lag.pop(0))
                    pv_lag.append((p, c, g, pt, oq))
                    if g >= FILLER_START and fi < len(fillers):
                        fillers[fi]()
                        fi += 1
                while fi < len(fillers):
                    fillers[fi]()
                    fi += 1
                return oq

            # ---- software pipeline over pairs ----
            alloc_pair(0)
            # prologue loads: k/q quarter 0 on the SP queue; the rest via the
            # otherwise-idle Pool engine queue (SP SEQ serializes DMA dispatch
            # at 565ns each; Pool dispatches in ~36ns)
            # spread prologue dispatch across idle engine queues: each DMA
            # costs ~565-667ns of serial sequencer time on its issuing engine
            load_k(0, 0)()
            load_q(0, 0)()
            load_k(0, 1)()
            load_v(0, 0)()
            load_k(0, 2)()
            load_k(0, 3)()
            load_v(0, 1)()
            # warm the PE clock gate with dummy matmuls while the loads fly
            warm = oq_pool.tile([128, 4, D + 1], F32, tag="oq", name="warm")
            for w in range(10):
                nc.tensor.matmul(
                    warm[:, 0, :], ident[:, 0:128], ident[:, 0 : D + 1],
                    start=True, stop=True,
                )

            oq_prev = {}
            for p in range(npair):
                nxt = p + 1 if p + 1 < npair else None
                if nxt is not None and p == 0:
                    alloc_pair(nxt)
                for c in range(NCHUNK):
                    if p == 0 and c == 0:
                        fillers = [
                            load_q(0, 1),
                            load_q(0, 2),
                            load_q(0, 3),
                        ]
                        if nxt is not None:
                            fillers += [load_k(nxt, 0), load_k(nxt, 1)]
                    else:
                        prev = (p, c - 1) if c > 0 else (p - 1, NCHUNK - 1)
                        fillers = list(epilogue(*prev, oq_prev[prev]))
                        if nxt is not None:
                            if c == 0:
                                if p > 0:
                                    alloc_pair(nxt)
                                fillers += [load_k(nxt, 0), load_k(nxt, 1)]
                            elif c == 1:
                                fillers += [load_k(nxt, 2), load_k(nxt, 3)]
                            elif c == 2:
                                fillers += [
                                    load_q(nxt, 0),
                                    load_q(nxt, 1),
                                    load_v(nxt, 0),
                                ]
                            else:
                                fillers += [
                                    load_q(nxt, 2),
                                    load_q(nxt, 3),
                                    load_v(nxt, 1),
                                ]
                    oq_prev[(p, c)] = emit_chunk(p, c, fillers)
            while pv_lag:
                emit_pv(*pv_lag.pop(0))
            for piece in epilogue(
                npair - 1, NCHUNK - 1, oq_prev[(npair - 1, NCHUNK - 1)]
            ):
                piece()
            pairs.clear()
    nc.finalize()
    return nc


def _host_prep(k, q, v, Wk, Wq, Wv):
    m1 = ((Wq.T @ Wk) / np.sqrt(np.float32(D))).astype(np.float32)

    def split_heads(x):
        return x.reshape(B, S, N_HEADS, D).transpose(0, 2, 1, 3).reshape(-1, S, D)

    qh = split_heads(q)
    kh = split_heads(k)
    vh = split_heads(v)

    import ml_dtypes

    E4 = ml_dtypes.float8_e4m3
    qt = (qh @ m1).astype(np.float32) * np.float32(QS)  # [32, S, D]

    def hi_lo(x):
        hi = x.astype(E4)
        lo = (x - hi.astype(np.float32)).astype(E4)
        return hi, lo

    def dsplit(x):  # [P, S, D] -> [P, 32, 2, S]: [p, t] <- d = t*32 + p
        return x.transpose(0, 2, 1).reshape(-1, 2, 32, S).transpose(0, 2, 1, 3)

    q_hi, q_lo = hi_lo(qt)
    k_hi, k_lo = hi_lo(kh.astype(np.float32))
    qs = np.zeros((B * N_HEADS, 97, 2, S), dtype=E4)
    ks = np.zeros((B * N_HEADS, 97, 2, S), dtype=E4)
    qs[:, 0:32] = dsplit(q_hi.astype(np.float32))
    qs[:, 32:64] = dsplit(q_lo.astype(np.float32))
    qs[:, 64:96] = dsplit(q_hi.astype(np.float32))
    ks[:, 0:32] = dsplit(k_hi.astype(np.float32))
    ks[:, 32:64] = dsplit(k_hi.astype(np.float32))
    ks[:, 64:96] = dsplit(k_lo.astype(np.float32))
    qs[:, 96, 0, :] = E4(10.0)
    qs[:, 96, 1, :] = E4(2.875)
    ks[:, 96, 0, :] = E4(16.0)
    ks[:, 96, 1, :] = E4(2.0)

    vp = (vh @ Wv.T).astype(np.float16)  # [32, S, D]
    vs = np.empty((B * N_HEADS, 128, NT, D + 1), dtype=np.float16)
    vs[:, :, :, :D] = vp.reshape(-1, NT, 128, D).transpose(0, 2, 1, 3)
    vs[:, :, :, D] = np.float16(1.0)
    return qs, ks, vs


def kernel(k, q, v, Wk, Wq, Wv):
    k = np.asarray(k, dtype=np.float32)
    q = np.asarray(q, dtype=np.float32)
    v = np.asarray(v, dtype=np.float32)
    Wk = np.asarray(Wk, dtype=np.float32)
    Wq = np.asarray(Wq, dtype=np.float32)
    Wv = np.asarray(Wv, dtype=np.float32)

    qs, ks, vs = _host_prep(k, q, v, Wk, Wq, Wv)

    if "nc" not in _NC_CACHE:
        _NC_CACHE["nc"] = build_nc()
    nc = _NC_CACHE["nc"]

    in_maps = []
    for c in range(N_CORES):
        sl = slice(c * NPAIR, (c + 1) * NPAIR)
        in_maps.append({"qs": qs[sl], "ks": ks[sl], "vs": vs[sl]})

    res = run_bass_kernel_spmd(nc, in_maps, core_ids=list(range(N_CORES)))
    outs = np.stack([r["out"] for r in res.results])  # [8, NPAIR, 128, NT, D]
    outs = outs.reshape(B, N_HEADS, 128, NT, D).transpose(0, 3, 2, 1, 4)
    out = outs.reshape(B, S, D_EMBED)  # s = t*128 + r
    return out
